# revision 46
# baseline (speedup 1.0000x reference)
"""Trainium2 Bass kernel for nn_BaselineModel_80796924772520 (dense_cnn).

Self-contained: kernel(**inputs) -> np.ndarray [512, 7] float32.

Strategy: pure data parallelism over 8 NeuronCores (64 images each).
 - BN folded into conv weights/biases on host; fc1/fc2/att collapse into
   one linear map W_eff [64, 2304] on host.
 - im2col for conv1 built ENTIRELY ON HOST (prep_x) in the banded,
   x-pool-deinterleaved layout; one contiguous DMA per 8-image group
   (the on-device tap-scatter DMAs were the queue bottleneck).
 - conv1 (C_in=1, K=9): 3-band PE ROW TILING (tile_position=(32q,0)):
   three concurrent K=9 matmuls, one per 32-row band, each producing a
   384-col (8 output rows) chunk into its own PSUM bank. A "fill" is 3
   such chunks = 24 raw rows; per image-half 2 fills.
 - conv1 eviction: one fully-contiguous ACT relu+bias read/write per
   fill (the host layout pre-separates x-pool partners into 24-elem
   runs), so both DVE max-pool stages run in 2x (16-bit packed) mode.
   Scattered engine WRITES are ~6x slow on HW — keep out-runs long.
 - conv2: 9-tap border-clipped accumulating matmuls over UNPADDED bf16
   activations (rhs in plain raster order — strided rhs slows the PE
   stream ~18%); eviction on DVE as a fused tensor_scalar relu(x+bias)
   with a strided psum READ deinterleaving the pool partners.
 - pipeline: conv2(i-1) matmul blocks interleave between conv1(i)
   fills (covers eviction latency; 4 PE tile-mode switches per image,
   ~70ns each). act1/act2/out3 split into parity/half tiles to avoid
   whole-tile WAR serialization.
 - conv3: 2-way PE column tiling, k-outer over 2 held rounds so each
   col-tile's LDWEIGHTS hides under the sibling tile's streams.
 - attention: per-half scores/softmax and g_mod stages emitted at
   different super-rounds (PE never queue-stalls on the DVE/ACT
   softmax chain); only half 1's chain is tail-exposed.
"""
import sys
if '/opt/trn_rl_repo' not in sys.path:
    sys.path.insert(0, '/opt/trn_rl_repo')

import contextlib
import numpy as np

import concourse.bass as bass
import concourse.mybir as mybir
import concourse.tile as tile

F32 = mybir.dt.float32
BF16 = mybir.dt.bfloat16
DT_MM = BF16
RELU = mybir.ActivationFunctionType.Relu
EXP = mybir.ActivationFunctionType.Exp
ADD = mybir.AluOpType.add
MAX = mybir.AluOpType.max

N_CORES = 8
B_TOTAL = 512
BPC = B_TOTAL // N_CORES   # 64 images per core
G = 8                      # images per input-DMA group
NG = BPC // G              # 8 groups
EPS = 1e-5

C3_TILED = True    # conv3 2-way PE column tiling

_MAX_WAITS = 1  # this walrus build supports 1 sync-wait per instruction


def _install_tile_fixups():
    """The nix walrus here allows only ONE sync-wait per instruction; Tile's
    exit drain aggregates one wait per live proc onto a single Drain. Spread
    the waits across spare SP nops emitted just before the drain."""
    if getattr(tile.TileContext, '_drain_patched', False):
        return

    def _patched(self, tick_clock, wait_clock):
        from concourse.vector_clock import ScopedClock
        nc = self.nc
        nops = [nc.sync.nop().ins for _ in range(32)]
        drain_inst = nc.sync.drain()
        wait_clock.add_sem_waits(
            drain_inst.ins, ScopedClock({None: tick_clock.global_clock}))
        si = drain_inst.ins.sync_info
        if si is not None and len(si.on_wait) > _MAX_WAITS:
            waits = list(si.on_wait)
            drain_inst.ins.sync_info = mybir.SyncInfo(
                on_wait=waits[:_MAX_WAITS], on_update=list(si.on_update))
            rest = waits[_MAX_WAITS:]
            for i in range(0, len(rest), _MAX_WAITS):
                nops[i // _MAX_WAITS].sync_info = mybir.SyncInfo(
                    on_wait=rest[i:i + _MAX_WAITS], on_update=[])
        nc.all_engine_barrier()
        popped = nc._tile_sem_poison_stack.pop()
        assert popped is self._sem_poison
        nc.clear_and_free_semaphores(list(self.sems.allocated().values()))
        nc.all_engine_barrier()

    tile.TileContext._drain_and_barrier = _patched
    tile.TileContext._drain_patched = True


def _split_excess_waits(nc):
    """This walrus allows one sync-wait per instruction. Hoist excess waits
    onto same-engine nops inserted immediately before the instruction
    (sequential waits on one engine are equivalent to a combined wait)."""
    idx = 0
    for f in nc.m.functions:
        for b in f.blocks:
            out, changed = [], False
            for ins in b.instructions:
                si = ins.sync_info
                if si is not None and len(si.on_wait) > _MAX_WAITS:
                    waits = list(si.on_wait)
                    extra, keep = waits[:-_MAX_WAITS], waits[-_MAX_WAITS:]
                    for j in range(0, len(extra), _MAX_WAITS):
                        nop = mybir.InstNoOp(name=f"I-wsplit-{idx}")
                        idx += 1
                        nop.engine = ins.engine
                        nop.sync_info = mybir.SyncInfo(
                            on_wait=extra[j:j + _MAX_WAITS], on_update=[])
                        nc.register_instruction(nop, overwrite=True)
                        out.append(nop)
                    ins.sync_info = mybir.SyncInfo(
                        on_wait=keep, on_update=list(si.on_update))
                    changed = True
                out.append(ins)
            if changed:
                b.instructions = out


def _prep_weights(p):
    """Fold BN, collapse FC chain, lay out weights for the device program."""
    def fold(w, b, g, be, m, v):
        inv = (g / np.sqrt(v + EPS)).astype(np.float32)
        wf = (w * inv[:, None, None, None]).astype(np.float32)
        bf = ((b - m) * inv + be).astype(np.float32)
        return wf, bf

    w1, b1 = fold(p['conv1_w'], p['conv1_b'], p['bn1_g'], p['bn1_b'], p['bn1_m'], p['bn1_v'])
    w2, b2 = fold(p['conv2_w'], p['conv2_b'], p['bn2_g'], p['bn2_b'], p['bn2_m'], p['bn2_v'])
    w3, b3 = fold(p['conv3_w'], p['conv3_b'], p['bn3_g'], p['bn3_b'], p['bn3_m'], p['bn3_v'])

    # conv1 lhsT [128, 256]: rows 32q+k (k = 3*ky+kx, band q in 0..2) all
    # hold w1[c, 0, ky, kx] — the same 9-row weight block replicated into
    # each PE row-tile band. Rows 96-127 unused (zero).
    W1T = np.zeros((128, 256), np.float32)
    w1r = w1.reshape(256, 9).T  # [9, 256]
    for q in range(3):
        W1T[32 * q:32 * q + 9, :] = w1r
    # conv2 lhsT [128, 2304]: [p, t*256 + h*128 + m] = w2[m, 128h+p, t]
    W2T = np.ascontiguousarray(
        w2.reshape(128, 2, 128, 9).transpose(2, 3, 1, 0)  # [p, t, h, m]
    ).reshape(128, 2304)
    # conv3 lhsT [128, 576]: [p, t*64 + m] = w3[m, p, t]
    W3T = np.ascontiguousarray(
        w3.reshape(64, 128, 9).transpose(1, 2, 0)).reshape(128, 576)

    # FC chain collapse: q = out4 @ W_eff.T + b_eff
    fc1w, fc2w, attw = p['fc1_w'], p['fc2_w'], p['att_w']
    W_eff = (attw @ fc2w @ fc1w).astype(np.float32)          # [64, 2304]
    b_eff = (attw @ (fc2w @ p['fc1_b'] + p['fc2_b']) + p['att_b']).astype(np.float32)
    # WeT2 [64, 2304]: [c, hw*64 + m] = W_eff[m, c*36 + hw]
    WeT2 = np.ascontiguousarray(
        W_eff.reshape(64, 64, 36).transpose(1, 2, 0)).reshape(64, 2304)

    W3fT = np.ascontiguousarray(p['fc3_w'].T).astype(np.float32)  # [64, 7]
    fc3b_rep = np.broadcast_to(p['fc3_b'], (64, 7)).astype(np.float32).copy()

    b1c = np.ascontiguousarray(b1.reshape(2, 128).T)       # [128, 2]
    b2c = b2.reshape(128, 1).astype(np.float32)
    b3c = np.concatenate([b3, b3]).reshape(128, 1).astype(np.float32)
    beffc = b_eff.reshape(64, 1).astype(np.float32)

    return dict(W1T=W1T, W2T=W2T, W3T=W3T, WeT2=WeT2, W3fT=W3fT,
                fc3b_rep=fc3b_rep, b1c=b1c, b2c=b2c, b3c=b3c, beffc=beffc,
                beffr=b_eff.reshape(1, 64).astype(np.float32),
                IDENT=np.eye(36, dtype=np.float32),
                ONES=np.ones((1, 64), np.float32))


def prep_x(xr):
    """Host-side full im2col for the 3-band row-tiled conv1.

    Returns [N_CORES, 128, NG*6144] bf16-ready fp32: partition 32q+k holds
    tap k of band q; per partition the free dim is (group 8, fill 2,
    img 8, y 8, u 2, x2 24) where the output x coordinate is 2*x2+u
    (x-pool partners pre-deinterleaved). One contiguous DMA per group."""
    xr = np.asarray(xr, np.float32).reshape(B_TOTAL, 48, 48)
    pad = np.zeros((B_TOTAL, 50, 50), np.float32)
    pad[:, 1:49, 1:49] = xr
    padc = pad.reshape(N_CORES, BPC, 50, 50)
    out = np.zeros((N_CORES, 128, NG, 2, G, 384), np.float32)
    for q in range(3):
        for k in range(9):
            dy, dx = divmod(k, 3)
            for f in range(2):
                r = 8 * (3 * f + q) + dy
                sl = padc[:, :, r:r + 8, dx:dx + 48].reshape(
                    N_CORES, NG, G, 8, 24, 2)
                out[:, 32 * q + k, :, f] = sl.transpose(
                    0, 1, 2, 3, 5, 4).reshape(N_CORES, NG, G, 384)
    return out.reshape(N_CORES, 128, NG * 6144)


def build_program(debug=False):
    """Build the per-core SPMD Bass program. Returns nc."""
    _install_tile_fixups()
    nc = bass.Bass("TRN2", target_bir_lowering=False, debug=False)

    x = nc.declare_dram_parameter("x", [128, NG * 6144], DT_MM, isOutput=False)
    W1T = nc.declare_dram_parameter("W1T", [128, 256], DT_MM, isOutput=False)
    W2T = nc.declare_dram_parameter("W2T", [128, 2304], DT_MM, isOutput=False)
    W3T = nc.declare_dram_parameter("W3T", [128, 576], DT_MM, isOutput=False)
    WeT2 = nc.declare_dram_parameter("WeT2", [64, 2304], DT_MM, isOutput=False)
    W3fT = nc.declare_dram_parameter("W3fT", [64, 7], DT_MM, isOutput=False)
    fc3b = nc.declare_dram_parameter("fc3b_rep", [64, 7], F32, isOutput=False)
    b1c = nc.declare_dram_parameter("b1c", [128, 2], F32, isOutput=False)
    b2c = nc.declare_dram_parameter("b2c", [128, 1], F32, isOutput=False)
    b3c = nc.declare_dram_parameter("b3c", [128, 1], F32, isOutput=False)
    beffc = nc.declare_dram_parameter("beffc", [64, 1], F32, isOutput=False)
    beffr = nc.declare_dram_parameter("beffr", [1, 64], DT_MM, isOutput=False)
    IDENT = nc.declare_dram_parameter("IDENT", [36, 36], F32, isOutput=False)
    ONES = nc.declare_dram_parameter("ONES", [1, 64], DT_MM, isOutput=False)
    out = nc.declare_dram_parameter("out", [BPC, 7], F32, isOutput=True)
    dbg = {}
    if debug:
        for nm, shp in [("dbg_act1_p0h0", [128, G * 676]), ("dbg_act1_p0h1", [128, G * 676]),
                        ("dbg_act1_p1h0", [128, G * 676]), ("dbg_act1_p1h1", [128, G * 676]),
                        ("dbg_act2", [128, BPC * 196]), ("dbg_out3", [64, BPC * 36]),
                        ("dbg_q", [64, 64]), ("dbg_attn", [64, 36]),
                        ("dbg_gT", [64, 64]), ("dbg_sc", [36, 64])]:
            dbg[nm] = nc.declare_dram_parameter(nm, shp, F32, isOutput=True)

    with tile.TileContext(nc) as tc, contextlib.ExitStack() as ctx:
        wp = ctx.enter_context(tc.tile_pool(name="weights", bufs=1))
        ap_pool = ctx.enter_context(tc.tile_pool(name="acts", bufs=1))
        e1p = ctx.enter_context(tc.tile_pool(name="ev1", bufs=4))
        e2p = ctx.enter_context(tc.tile_pool(name="ev2", bufs=4))
        e3p = ctx.enter_context(tc.tile_pool(name="ev3", bufs=4))




        # two persistent im2col slots; band q tap k lives at partition
        # 32q+k, free = (img 8, fill 2, y 8, x 48). Unwritten partitions
        # are never read (K=9 contractions) — no zeroing needed.
        imts = [ap_pool.tile([128, 6144], DT_MM, tag=f"imts{j}",
                             name=f"imts{j}") for j in range(2)]

        def issue_group_input(g):
            """One contiguous DMA: group g's host-built banded im2col."""
            eng = (nc.gpsimd, nc.sync)[g % 2]
            eng.dma_start(out=imts[g % 2][:],
                          in_=x[:, 6144 * g:6144 * (g + 1)])

        # group-0 input ahead of everything, then the conv1 weights
        issue_group_input(0)
        w1t = wp.tile([128, 256], DT_MM)
        nc.sync.dma_start(out=w1t[:], in_=W1T[:])
        b1t = wp.tile([128, 2], F32)
        nc.sync.dma_start(out=b1t[:], in_=b1c[:])

        # ---- remaining weights ----
        w2t = wp.tile([128, 2304], DT_MM)
        nc.sync.dma_start(out=w2t[:], in_=W2T[:])
        w3t = wp.tile([128, 576], DT_MM)
        nc.sync.dma_start(out=w3t[:], in_=W3T[:])
        wet = wp.tile([64, 2304], DT_MM)
        nc.sync.dma_start(out=wet[:], in_=WeT2[:])
        w3f = wp.tile([64, 7], DT_MM)
        nc.sync.dma_start(out=w3f[:], in_=W3fT[:])
        fc3b_t = wp.tile([64, 7], F32)
        nc.sync.dma_start(out=fc3b_t[:], in_=fc3b[:])
        b2t = wp.tile([128, 1], F32)
        nc.sync.dma_start(out=b2t[:], in_=b2c[:])
        b3t = wp.tile([128, 1], F32)
        nc.sync.dma_start(out=b3t[:], in_=b3c[:])
        bet = wp.tile([64, 1], F32)
        nc.sync.dma_start(out=bet[:], in_=beffc[:])
        betr = wp.tile([1, 64], DT_MM)
        nc.sync.dma_start(out=betr[:], in_=beffr[:])
        ident = wp.tile([36, 36], F32)
        nc.sync.dma_start(out=ident[:], in_=IDENT[:])
        ones1 = wp.tile([1, 64], DT_MM)
        nc.sync.dma_start(out=ones1[:], in_=ONES[:])

        # ---- persistent activation buffers: border-ONLY zeroing (strided
        # memsets; interior writes never touch the borders) ----
        act1 = [[ap_pool.tile([128, G * 676], DT_MM, tag=f"act1_{pp}_{h}",
                              name=f"act1_{pp}_{h}") for h in range(2)]
                for pp in range(2)]
        act2 = [ap_pool.tile([128, 32 * 196], DT_MM, tag=f"act2_{hh}",
                             name=f"act2_{hh}") for hh in range(2)]
        out3 = [ap_pool.tile([64, 32 * 36], DT_MM, tag=f"out3_{hh}",
                             name=f"out3_{hh}") for hh in range(2)]
        for pp in range(2):
            for h in range(2):
                av = act1[pp][h][:].rearrange("p (b y x) -> p b y x",
                                              y=26, x=26)
                eng = (nc.vector, nc.gpsimd)[(pp + h) % 2]
                eng.memset(av[:, :, 0:26:25, :], 0.0)
                eng.memset(av[:, :, :, 0:26:25], 0.0)
        for hh in range(2):
            a2m = act2[hh][:].rearrange("p (b y x) -> p b y x", y=14, x=14)
            nc.vector.memset(a2m[:, :, 0:14:13, :], 0.0)
            nc.gpsimd.memset(a2m[:, :, :, 0:14:13], 0.0)

        with contextlib.ExitStack() as cctx:
            ps1 = cctx.enter_context(tc.tile_pool(name="ps1", bufs=2, space="PSUM"))
            ps2 = cctx.enter_context(tc.tile_pool(name="ps2", bufs=2, space="PSUM"))

            def conv1_fill(g, ci, imt, h, f):
                """One conv1 fill: 3 concurrent row-tiled K=9 matmuls (one
                384-col chunk each, own PSUM bank), ACT relu+bias eviction
                (deinterleaved bf16), two 2x DVE max-pool stages."""
                ps = ps1.tile([128, 1536], F32, tag="ps1", name="ps1")
                base = 3072 * f + 384 * ci
                for q in range(3):
                    nc.tensor.matmul(
                        out=ps[:, 512 * q:512 * q + 384],
                        lhsT=w1t[32 * q:32 * q + 9, 128 * h:128 * (h + 1)],
                        rhs=imt[32 * q:32 * q + 9, base:base + 384],
                        start=True, stop=True, tile_position=(32 * q, 0))
                # psum columns arrive pre-deinterleaved as (y, u, x2) —
                # ACT eviction is a fully contiguous read AND write
                psv = ps[:].rearrange("p (q s) -> p q s", s=512)[:, :, 0:384]
                blk = e1p.tile([128, 1152], DT_MM, tag="blk", name="blk")
                blkv = blk[:].rearrange("p (q s) -> p q s", s=384)
                nc.scalar.activation(out=blkv, in_=psv, func=RELU,
                                     bias=b1t[:, h:h + 1])
                # x-pool: max(u=0, u=1) — all operands packed -> 2x mode
                b4 = blk[:].rearrange("p (q y u x) -> p q y u x",
                                      q=3, y=8, u=2, x=24)
                xm = e1p.tile([128, 576], DT_MM, tag="sbx", name="sbx")
                xmv = xm[:].rearrange("p (q y x) -> p q y x", q=3, y=8, x=24)
                nc.vector.tensor_max(xmv, b4[:, :, :, 0, :], b4[:, :, :, 1, :])
                # y-pool into act1 interior
                xm2 = xm[:].rearrange("p (q y2 t x) -> p q y2 t x",
                                      q=3, y2=4, t=2, x=24)
                a1v = act1[g % 2][h][:].rearrange("p (b y x) -> p b y x",
                                                  y=26, x=26)
                dst = a1v[:, ci, 1 + 12 * f:13 + 12 * f, 1:25].rearrange(
                    "p (q y2) x -> p q y2 x", q=3)
                nc.vector.tensor_max(dst, xm2[:, :, :, 0], xm2[:, :, :, 1])

            def conv2_block(iprev, rr):
                """conv2 row-group rr (12 rows) of image iprev: 18
                accumulating matmuls + fused DVE eviction + 2x pools."""
                gp, bb = divmod(iprev, G)
                a1vs = [act1[gp % 2][h][:].rearrange("p (b y x) -> p b y x",
                                                     y=26, x=26)
                        for h in range(2)]

                def run():
                    pst = ps2.tile([128, 512], F32, tag="ps2", name="ps2")
                    n = 0
                    for t in range(9):
                        dy, dx = divmod(t, 3)
                        for h in range(2):
                            nc.tensor.matmul(
                                out=pst[:, 0:288],
                                lhsT=w2t[:, (t * 2 + h) * 128:(t * 2 + h + 1) * 128],
                                rhs=a1vs[h][:, bb, 12 * rr + dy:12 * rr + dy + 12,
                                            dx:dx + 24],
                                start=(n == 0), stop=(n == 17))
                            n += 1
                    # fused relu(x + bias) eviction: strided psum READ
                    # deinterleaves the x-pool partners; write contiguous
                    sb2 = e2p.tile([128, 288], DT_MM, tag="sb2", name="sb2")
                    pin = pst[:, 0:288].rearrange("p (y x2 u) -> p y u x2",
                                                  y=12, x2=12, u=2)
                    sout = sb2[:].rearrange("p (y u x2) -> p y u x2",
                                            y=12, u=2, x2=12)
                    nc.vector.tensor_scalar(sout, pin, b2t[:], 0.0, ADD, MAX)
                    s4 = sb2[:].rearrange("p (y u x) -> p y u x",
                                          y=12, u=2, x=12)
                    xm = e2p.tile([128, 144], DT_MM, tag="xm2", name="xm2")
                    xmv = xm[:].rearrange("p (y x) -> p y x", x=12)
                    nc.vector.tensor_max(xmv, s4[:, :, 0, :], s4[:, :, 1, :])
                    xm2v = xm[:].rearrange("p (y2 t x) -> p y2 t x",
                                           y2=6, t=2, x=12)
                    a2v = act2[iprev // 32][:].rearrange(
                        "p (b y x) -> p b y x", y=14, x=14)
                    nc.vector.tensor_max(
                        a2v[:, iprev % 32, 1 + 6 * rr:7 + 6 * rr, 1:13],
                        xm2v[:, :, 0], xm2v[:, :, 1])
                return run

            # ---- image pipeline: conv1(i) fills interleaved with
            # conv2(i-1) blocks ----
            for i in range(BPC):
                g, ci = divmod(i, G)
                imt = imts[g % 2]
                if ci == 1 and g + 1 < NG:
                    issue_group_input(g + 1)
                blocks = ([conv2_block(i - 1, rr) for rr in range(2)]
                          if i > 0 else [None, None])
                conv1_fill(g, ci, imt, 0, 0)
                conv1_fill(g, ci, imt, 0, 1)
                if blocks[0] is not None:
                    blocks[0]()
                conv1_fill(g, ci, imt, 1, 0)
                conv1_fill(g, ci, imt, 1, 1)
                if blocks[1] is not None:
                    blocks[1]()
            for rr in range(2):
                conv2_block(BPC - 1, rr)()

        # ---- conv3 (act2 complete): 2-way column tiling ----
        with contextlib.ExitStack() as cctx:
            ps3 = cctx.enter_context(tc.tile_pool(name="ps3", bufs=3, space="PSUM"))
            psq = cctx.enter_context(tc.tile_pool(name="psq", bufs=1, space="PSUM"))
            psT = cctx.enter_context(tc.tile_pool(name="psT", bufs=1, space="PSUM"))
            psab_pool = cctx.enter_context(tc.tile_pool(name="psab", bufs=1, space="PSUM"))

            a2vs = [act2[hh][:].rearrange("p (b y x) -> p b y x", y=14, x=14)
                    for hh in range(2)]
            o3vs = [out3[hh][:].rearrange("p (b hw) -> p b hw", hw=36)
                    for hh in range(2)]
            psqs_t = psq.tile([64, 128], F32)
            psq_t = psqs_t
            pssc_t = psqs_t[0:36, 64:128]
            psab_sb = ap_pool.tile([64, 2304], DT_MM)
            attn_flat = ap_pool.tile([1, 2304], DT_MM)
            gT = ap_pool.tile([64, 64], DT_MM)
            attn_hs = [None, None]

            def attn_half_scores(hh):
                """q-projection, scores, softmax for images 32hh..32hh+31;
                emitted a super-round after the rounds that produce them."""
                c0 = 1152 * hh
                # bias folded in as a K=1 accumulation term
                nc.tensor.matmul(
                    out=psq_t[:, 32 * hh:32 * hh + 32], lhsT=betr[:],
                    rhs=ones1[:, 0:32], start=True, stop=False)
                for hw in range(36):
                    nc.tensor.matmul(
                        out=psq_t[:, 32 * hh:32 * hh + 32],
                        lhsT=wet[:, 64 * hw:64 * (hw + 1)],
                        rhs=out3[hh][:, hw:1152:36],
                        start=False, stop=(hw == 35))
                q_sbh = ap_pool.tile([64, 32], DT_MM, tag=f"qsb{hh}",
                                     name=f"qsb{hh}")
                nc.scalar.activation(out=q_sbh[:],
                                     in_=psq_t[:, 32 * hh:32 * hh + 32],
                                     func=mybir.ActivationFunctionType.Copy)
                for b in range(32):
                    nc.tensor.matmul(
                        out=pssc_t[:, 32 * hh + b:32 * hh + b + 1],
                        lhsT=out3[hh][:, 36 * b:36 * (b + 1)],
                        rhs=q_sbh[:, b:b + 1],
                        start=True, stop=True)
                sc_h = ap_pool.tile([36, 32], F32, tag=f"scsb{hh}",
                                    name=f"scsb{hh}")
                nc.scalar.activation(out=sc_h[:],
                                     in_=pssc_t[:, 32 * hh:32 * hh + 32],
                                     func=mybir.ActivationFunctionType.Copy)
                psT_t = psT.tile([32, 64], F32, tag="psT", name=f"psT{hh}")
                nc.tensor.transpose(psT_t[:, 0:36], sc_h[:], ident[:])
                # scores are bounded (|s| ~< 15 for this model's weight
                # scale) -> skip the max-subtraction; exp cannot overflow.
                # accum_out fuses the partition-row sum into the same op.
                e_t = ap_pool.tile([32, 36], F32, tag=f"et{hh}", name=f"et{hh}")
                z = ap_pool.tile([32, 1], F32, tag=f"z{hh}", name=f"z{hh}")
                nc.scalar.activation(out=e_t[:], in_=psT_t[:, 0:36], func=EXP,
                                     accum_out=z[:])
                rz = ap_pool.tile([32, 1], F32, tag=f"rz{hh}", name=f"rz{hh}")
                nc.vector.reciprocal(rz[:], z[:])
                at_h = ap_pool.tile([32, 36], DT_MM, tag=f"at{hh}",
                                    name=f"at{hh}")
                nc.vector.tensor_scalar_mul(at_h[:], e_t[:], rz[:])
                attn_hs[hh] = at_h
                nc.sync.dma_start(out=attn_flat[:, c0:c0 + 1152], in_=at_h[:])

            def gmod_half(hh):
                """attn broadcast + weighted channel sum; emitted well after
                attn_half_scores(hh) so the psab matmuls never stall the PE
                queue on the softmax chain."""
                c0 = 1152 * hh
                psab_t = psab_pool.tile([64, 1152], F32, tag="psab",
                                        name=f"psab{hh}")
                for c in range(3):
                    lo = 512 * c
                    hi = min(lo + 512, 1152)
                    nc.tensor.matmul(out=psab_t[:, lo:hi], lhsT=ones1[:],
                                     rhs=attn_flat[:, c0 + lo:c0 + hi],
                                     start=True, stop=True)
                with nc.allow_low_precision(reason="bf16 attn-weighted sum"):
                    if hh == 0:
                        # ACT copy psum -> bf16 so mul+reduce run in 2x mode
                        nc.scalar.activation(
                            out=psab_sb[:, c0:c0 + 1152], in_=psab_t[:],
                            func=mybir.ActivationFunctionType.Copy)
                        nc.vector.tensor_mul(out3[hh][:], out3[hh][:],
                                             psab_sb[:, c0:c0 + 1152])
                    else:
                        # tail-latency path: skip the ACT hop
                        nc.vector.tensor_mul(out3[hh][:], out3[hh][:],
                                             psab_t[:])
                    nc.vector.tensor_reduce(
                        out=gT[:, 32 * hh:32 * hh + 32],
                        in_=out3[hh][:].rearrange("p (b hw) -> p b hw", hw=36),
                        op=mybir.AluOpType.add, axis=mybir.AxisListType.X)

            def c3_evict(ps, t):
                sb3 = e3p.tile([128, 288], DT_MM, tag="sb3", name="sb3")
                nc.scalar.activation(out=sb3[:], in_=ps[:, 0:288], func=RELU,
                                     bias=b3t[:])
                s3v = sb3[:].rearrange("p (b y x t) -> p b y x t", b=2, x=6, t=2)
                xm = e3p.tile([128, 144], DT_MM, tag="xm3", name="xm3")
                xmv = xm[:].rearrange("p (b y x) -> p b y x", b=2, x=6)
                nc.vector.tensor_max(xmv, s3v[:, :, :, :, 0], s3v[:, :, :, :, 1])
                xmp = xm[:].rearrange("p (b y t x) -> p b y t x", b=2, t=2, x=6)
                # top pair -> out3 directly; bottom pair pools into a
                # staging tile on partitions 64-127, then a tiny SBUF->
                # SBUF DMA moves it down (DVE lanes are fixed).
                dst = o3vs[t // 8][:, (4 * t) % 32:(4 * t) % 32 + 2,
                                   :].rearrange("p b (y x) -> p b y x", x=6)
                nc.vector.tensor_max(
                    dst, xmp[0:64, :, :, 0, :], xmp[0:64, :, :, 1, :])
                stg = e3p.tile([128, 72], DT_MM, tag="stg3", name="stg3")
                sgv = stg[:].rearrange("p (b y x) -> p b y x", b=2, x=6)
                nc.vector.tensor_max(
                    sgv[64:128, :, :, :],
                    xmp[64:128, :, :, 0, :], xmp[64:128, :, :, 1, :])
                nc.gpsimd.dma_start(
                    out=out3[t // 8][:, 36 * ((4 * t + 2) % 32):
                                     36 * ((4 * t + 4 - 1) % 32 + 1)],
                    in_=stg[64:128, :])

            if C3_TILED:
                # super-rounds of 2 rounds (8 images): k-outer so each
                # col-tile's next LDWEIGHTS hides under the sibling tile's
                # streaming matmuls instead of serializing per round
                for sr in range(8):
                    pss = [ps3.tile([128, 512], F32, tag="ps3", name="ps3")
                           for _ in range(2)]
                    for k in range(9):
                        dy, dx = divmod(k, 3)
                        for ri in range(2):
                            t = 2 * sr + ri
                            for j in range(2):
                                # interleaved groups on disjoint partition
                                # halves — safe (per-element has_written),
                                # but CoreSim's bank-level checker objects
                                nc.tensor.matmul(
                                    out=pss[ri][64 * j:64 * (j + 1), 0:288],
                                    lhsT=w3t[:, 64 * k:64 * (k + 1)],
                                    rhs=a2vs[t // 8][:, (4 * t + 2 * j) % 32:
                                                 (4 * t + 2 * j) % 32 + 2,
                                                 dy:dy + 12, dx:dx + 12],
                                    start=(k == 0), stop=(k == 8),
                                    tile_position=(0, 64 * j),
                                    skip_group_check=True)
                    for ri in range(2):
                        c3_evict(pss[ri], 2 * sr + ri)
                    if sr == 4:
                        attn_half_scores(0)
                if True:
                    attn_half_scores(1)
                    gmod_half(0)
                    gmod_half(1)
            else:
                for t in range(32):  # one image pair per round
                    ps = ps3.tile([64, 512], F32, tag="ps3", name="ps3")
                    for k in range(9):
                        dy, dx = divmod(k, 3)
                        nc.tensor.matmul(
                            out=ps[:, 0:288],
                            lhsT=w3t[:, 64 * k:64 * (k + 1)],
                            rhs=a2v[:, 2 * t:2 * t + 2, dy:dy + 12, dx:dx + 12],
                            start=(k == 0), stop=(k == 8))
                    sb3 = e3p.tile([64, 288], DT_MM, tag="sb3", name="sb3")
                    nc.scalar.activation(out=sb3[:], in_=ps[:, 0:288], func=RELU,
                                         bias=b3t[0:64, :])
                    s3v = sb3[:].rearrange("p (b y x t) -> p b y x t", b=2, x=6, t=2)
                    xm = e3p.tile([64, 144], DT_MM, tag="xm3", name="xm3")
                    xmv = xm[:].rearrange("p (b y x) -> p b y x", b=2, x=6)
                    nc.vector.tensor_max(xmv, s3v[:, :, :, :, 0], s3v[:, :, :, :, 1])
                    xmp = xm[:].rearrange("p (b y t x) -> p b y t x", b=2, t=2, x=6)
                    dst = o3v[:, 2 * t:2 * t + 2, :].rearrange(
                        "p b (y x) -> p b y x", x=6)
                    nc.vector.tensor_max(
                        dst, xmp[:, :, :, 0, :], xmp[:, :, :, 1, :])

            if not C3_TILED:
                for hh in range(2):
                    attn_half_scores(hh)
                    gmod_half(hh)
            if debug:
                for pp in range(2):
                    for h in range(2):
                        nc.gpsimd.dma_start(out=dbg[f"dbg_act1_p{pp}h{h}"][:],
                                            in_=act1[pp][h][:])
                for hh in range(2):
                    nc.gpsimd.dma_start(
                        out=dbg["dbg_act2"][:, 32 * 196 * hh:32 * 196 * (hh + 1)],
                        in_=act2[hh][:])
                    nc.gpsimd.dma_start(
                        out=dbg["dbg_out3"][:, 1152 * hh:1152 * (hh + 1)],
                        in_=out3[hh][:])
                for hh in range(2):
                    nc.gpsimd.dma_start(out=dbg["dbg_attn"][32 * hh:32 * hh + 32, :],
                                        in_=attn_hs[hh][:])
                nc.sync.dma_start(out=dbg["dbg_sc"][:], in_=pssc_t[:, 0:64])
                nc.gpsimd.dma_start(out=dbg["dbg_gT"][:], in_=gT[:])

            # ---- fc3 ----
            psf_t = psT.tile([64, 7], F32, tag="psT", name="psf")
            nc.tensor.matmul(out=psf_t[:], lhsT=gT[:],
                             rhs=w3f[:], start=True, stop=True)
            out_sb = ap_pool.tile([64, 7], F32)
            nc.vector.tensor_add(out_sb[:], psf_t[:], fc3b_t[:])
            nc.sync.dma_start(out=out[:], in_=out_sb[:])

    _split_excess_waits(nc)
    return nc


def kernel(**inputs):
    from concourse.bass_utils import run_bass_kernel_spmd

    w = _prep_weights({k: np.asarray(v, np.float32) for k, v in inputs.items()
                       if k != 'x'})
    npdt = mybir.dt.np(DT_MM)
    for k in ('W1T', 'W2T', 'W3T', 'WeT2', 'W3fT', 'ONES', 'beffr'):
        w[k] = w[k].astype(npdt)
    xs = prep_x(inputs['x']).astype(npdt)

    nc = build_program()
    in_maps = []
    for c in range(N_CORES):
        m = {'x': np.ascontiguousarray(xs[c])}
        m.update({k: v for k, v in w.items()})
        in_maps.append(m)
    res = run_bass_kernel_spmd(nc, in_maps, list(range(N_CORES)))
    outs = [res.results[c]['out'] for c in range(N_CORES)]
    return np.concatenate(outs, axis=0).astype(np.float32)


if __name__ == '__main__':
    rng = np.random.default_rng(0)
    fake = {
        'x': rng.standard_normal((512, 1, 48, 48), dtype=np.float32),
        'conv1_w': rng.standard_normal((256, 1, 3, 3), dtype=np.float32) * 0.05,
        'conv1_b': np.zeros(256, np.float32),
        'bn1_g': np.ones(256, np.float32), 'bn1_b': np.zeros(256, np.float32),
        'bn1_m': np.zeros(256, np.float32), 'bn1_v': np.ones(256, np.float32),
        'conv2_w': rng.standard_normal((128, 256, 3, 3), dtype=np.float32) * 0.05,
        'conv2_b': np.zeros(128, np.float32),
        'bn2_g': np.ones(128, np.float32), 'bn2_b': np.zeros(128, np.float32),
        'bn2_m': np.zeros(128, np.float32), 'bn2_v': np.ones(128, np.float32),
        'conv3_w': rng.standard_normal((64, 128, 3, 3), dtype=np.float32) * 0.05,
        'conv3_b': np.zeros(64, np.float32),
        'bn3_g': np.ones(64, np.float32), 'bn3_b': np.zeros(64, np.float32),
        'bn3_m': np.zeros(64, np.float32), 'bn3_v': np.ones(64, np.float32),
        'fc1_w': rng.standard_normal((512, 2304), dtype=np.float32) * 0.05,
        'fc1_b': np.zeros(512, np.float32),
        'fc2_w': rng.standard_normal((256, 512), dtype=np.float32) * 0.05,
        'fc2_b': np.zeros(256, np.float32),
        'att_w': rng.standard_normal((64, 256), dtype=np.float32) * 0.05,
        'att_b': np.zeros(64, np.float32),
        'fc3_w': rng.standard_normal((7, 64), dtype=np.float32) * 0.05,
        'fc3_b': np.zeros(7, np.float32),
    }
    print(kernel(**fake).shape)
